# revision 69
# baseline (speedup 1.0000x reference)
"""Multi-head attention (B=8, N=1024, D=768, H=12) on 8 TRN2 NeuronCores.

Data-parallel: one batch element per core, no collectives. Per-core kernel:
  xT = x.T                          (PE transpose, bf16)
  qkT[c, t] = w_qkv[:, c].T @ xT    (c in [0, 1536): q rows then k rows,
                                     bias fused, q pre-scaled by 1/8)
  v[t, c]   = xT.T @ w_qkv[:, 1536:]  (bias folded later: softmax rows
                                       sum to 1 so attn @ (v+b) = attn@v + b)
  per head pair: scoresT[k, q] = kT.T @ qT for both heads as ADJACENT
            64-row-mode matmuls (PE tiles T0/T8 run concurrently, ~2x);
            exp over [128,1024] psum in one wide ACTIVATE per (kc, par);
            aT+sums = vpack.T @ expT row-split into T0/T8 accumulation
            chains (key 64-chunks alternate SBUF partition halves), then
            lo+hi combined on DVE; ones column in vpack emits softmax
            denominators; aT = (lo+hi) * (1/sums) + b_v.
  out = aT.T @ w_proj + b_proj

All matmul operands bf16 (PSUM fp32, softmax fp32). Engine balance:
exp stream ~97us on scalar, PE ~110us (scores/attnv at 2x via array
tiling), DVE ~100us, staging casts on GPSIMD.
"""

import os

import numpy as np

import bass_rust
from bass_rust import ScopedClock

import concourse.bass as bass
import concourse.tile as tile
from concourse import mybir
from concourse.bass_utils import run_bass_kernel_spmd
from concourse.masks import make_identity

# ---------------------------------------------------------------------------
# Workarounds: this container's walrus allows only ONE sync wait per
# instruction ("Too many sync wait commands"). Split extras onto same-engine
# NoOps (engine sequencers execute in program order).
# ---------------------------------------------------------------------------
_MAX_WAITS = 1


def _patched_drain_and_barrier(self, tick_clock, wait_clock):
    nc = self.nc
    drain_inst = nc.sync.drain()
    wait_clock.add_sem_waits(
        drain_inst.ins, ScopedClock({None: tick_clock.global_clock})
    )
    waits = list(drain_inst.ins.sync_info.on_wait)
    if len(waits) > _MAX_WAITS:
        drain_inst.ins.sync_info = bass_rust.SyncInfo(on_wait=[], on_update=[])
        by_num = {h.num: h for h in self.sems.allocated().values()}
        for w in waits:
            h = by_num.get(w.id)
            if h is None:
                h = bass_rust.SemaphoreHandle(name=w.ant_name, num=w.id)
            nc.sync.wait_ge(h, w.wait_value)

    nc.all_engine_barrier()
    assert self.sems is not None
    popped = nc._tile_sem_poison_stack.pop()
    assert popped is self._sem_poison
    nc.clear_and_free_semaphores(list(self.sems.allocated().values()))
    nc.all_engine_barrier()


tile.TileContext._drain_and_barrier = _patched_drain_and_barrier


def _legalize_waits(nc):
    n_split = 0
    for fn in nc.m.functions:
        for bb in fn.blocks:
            insts = bb.instructions
            if not any(
                i.sync_info is not None and len(i.sync_info.on_wait) > _MAX_WAITS
                for i in insts
            ):
                continue
            new = []
            for inst in insts:
                si = inst.sync_info
                if si is not None and len(si.on_wait) > _MAX_WAITS:
                    waits = list(si.on_wait)
                    keep, extra = waits[:_MAX_WAITS], waits[_MAX_WAITS:]
                    for j, w in enumerate(extra):
                        nop = mybir.InstNoOp(
                            name=f"{inst.name}-ws{j}", ins=[], outs=[],
                            engine=inst.engine,
                        )
                        nop.sync_info = bass_rust.SyncInfo(on_wait=[w], on_update=[])
                        new.append(nop)
                        n_split += 1
                    inst.sync_info = bass_rust.SyncInfo(
                        on_wait=keep, on_update=list(si.on_update)
                    )
                new.append(inst)
            bb.instructions = new
    return n_split


def _dedupe_ldweights(nc):
    """Drop an InstLdweights when the PE already holds identical weights
    (same physical AP + mode). bass emits one LDW per matmul; back-to-back
    matmuls sharing a stationary operand reload it pointlessly."""
    n = 0
    for fn in nc.m.functions:
        for bb in fn.blocks:
            insts = bb.instructions
            last_key = None
            new = []
            changed = False
            for inst in insts:
                tn = type(inst).__name__
                if tn == "InstLdweights":
                    key = (
                        repr(inst.ins[0]),
                        getattr(inst, "perf_mode", None),
                        getattr(inst, "is_transpose", None),
                        getattr(inst, "tile_position", None),
                    )
                    if key == last_key:
                        si = inst.sync_info
                        if si is not None and (si.on_wait or si.on_update):
                            nop = mybir.InstNoOp(
                                name=inst.name + "-dw", ins=[], outs=[],
                                engine=inst.engine,
                            )
                            nop.sync_info = si
                            new.append(nop)
                        n += 1
                        changed = True
                        continue
                    last_key = key
                new.append(inst)
            if changed:
                bb.instructions = new
    return n


# ---------------------------------------------------------------------------
# Kernel builder (per-core shapes hardcoded: x [1024, 768])
# ---------------------------------------------------------------------------
N, D, H, HD = 1024, 768, 12, 64
NT = N // 128       # 8 token chunks
DC = D // 128       # 6 d chunks
CT = (2 * D) // 128  # 12 qk col tiles
KC = N // 128       # 8 key chunks
SCALE = HD ** -0.5

F32 = mybir.dt.float32
BF16 = mybir.dt.bfloat16
Exp = mybir.ActivationFunctionType.Exp
ADD = mybir.AluOpType.add
MULT = mybir.AluOpType.mult


def build(legalize=True):
    nc = bass.Bass()
    x_d = nc.declare_dram_parameter("x", [N, D], F32, isOutput=False)
    wqkv_d = nc.declare_dram_parameter("w_qkv", [D, 3 * D], F32, isOutput=False)
    bqkv_d = nc.declare_dram_parameter("b_qkv", [3 * D], F32, isOutput=False)
    wp_d = nc.declare_dram_parameter("w_proj", [D, D], F32, isOutput=False)
    bp_d = nc.declare_dram_parameter("b_proj", [D], F32, isOutput=False)
    out_d = nc.declare_dram_parameter("out", [N, D], F32, isOutput=True)

    with tile.TileContext(nc) as tc:
        with (
            tc.tile_pool(name="persist", bufs=1) as persist,
            tc.tile_pool(name="consts", bufs=1) as consts,
            tc.tile_pool(name="pstage", bufs=3) as pstage,
            tc.tile_pool(name="wstage", bufs=2) as wstage,
            tc.tile_pool(name="ppa", bufs=4) as ppa,
            tc.tile_pool(name="prp", bufs=4) as prp,
            tc.tile_pool(name="pbc", bufs=4) as pbc,
            tc.tile_pool(name="plo", bufs=1) as plo,
            tc.tile_pool(name="pexp", bufs=3) as pexp,
            tc.tile_pool(name="drp", bufs=2, space="DRAM") as drp,
        ):
            qkT = persist.tile([128, CT, N], BF16)         # [qk col, tok]
            vpack = persist.tile([128, KC, 6, 2, 128], BF16)
            aT = persist.tile([128, DC, N], BF16)          # [d, tok]
            wp_sb = persist.tile([128, DC, D], BF16)
            xT = persist.tile([128, DC, NT, 128], BF16)    # [d, tok]
            wqk = persist.tile([128, DC, 3 * D], BF16)
            bqk_sb = consts.tile([128, CT], F32)
            bv_sb = consts.tile([128, DC], F32)
            bproj_bc = consts.tile([128, D], F32)
            ident = consts.tile([128, 128], BF16)

            make_identity(nc, ident[:])

            # ---- load phase: x half 0 + pair-0 qk weight cols first ----
            # (critical path to the first exp), then x half 1, w_v, the
            # remaining qk columns and w_proj (all land under compute).
            WQK0 = ((0, 256, 0), (768, 1024, 256))         # pair-0 q,k cols
            W2 = ((256, 768, 0), (1024, 1536, 512))        # pairs 1-5 cols

            xbfs = []

            def load_x_chunk(i):
                xst = pstage.tile([128, D], F32, tag="xst")
                nc.sync.dma_start(
                    xst[:], x_d.ap()[i * 128 : (i + 1) * 128, :]
                )
                xbf = pstage.tile([128, D], BF16, tag="xbf")
                nc.vector.tensor_copy(xbf[:], xst[:])
                xbfs.append(xbf)

            for i in range(4):
                load_x_chunk(i)
            wq_ap = wqkv_d.ap()
            for kc in range(DC):
                wst0 = wstage.tile([128, 512], F32, tag="wst0", name="wst0")
                # q cols 0:256 + k cols 768:1024 in one 2-block strided DMA
                nc.sync.dma_start(
                    wst0[:],
                    bass.AP(tensor=wq_ap.tensor,
                            offset=wq_ap.offset + kc * 128 * 3 * D,
                            ap=[[3 * D, 128], [768, 2], [1, 256]]),
                )
                for c0, c1, p0 in WQK0:
                    nc.vector.tensor_copy(
                        wqk[:, kc, c0:c1], wst0[:, p0 : p0 + (c1 - c0)]
                    )
            for i in range(4, NT):
                load_x_chunk(i)

            nc.sync.dma_start(
                bqk_sb[:], bqkv_d.ap()[: 2 * D].rearrange("(o i) -> i o", i=128)
            )
            nc.sync.dma_start(
                bv_sb[:], bqkv_d.ap()[2 * D :].rearrange("(o i) -> i o", i=128)
            )
            bp_ap = bp_d.ap()
            nc.sync.dma_start(
                bproj_bc[:],
                bass.AP(tensor=bp_ap.tensor, offset=bp_ap.offset,
                        ap=[[0, 128]] + bp_ap.ap),
            )
            # ones columns of vpack (even head: col 127; odd head: col 0);
            # the v-value columns are fully overwritten by the v copies and
            # the remaining columns feed only unread psum partitions.
            nc.vector.memset(vpack[:, :, :, 0, 127:128], 1.0)
            nc.vector.memset(vpack[:, :, :, 1, 0:1], 1.0)

            # x transposes (PE warms up here); batched psum->sbuf copies on
            # the scalar engine, which is idle until the first exp.
            def emit_transposes(ps_tp, ts):
                for t in ts:
                    for d0, d1 in ((0, 4), (4, 6)):
                        tp = ps_tp.tile([128, (d1 - d0) * 128], BF16, tag="tp")
                        for j, dc in enumerate(range(d0, d1)):
                            nc.tensor.transpose(
                                tp[:, j * 128 : (j + 1) * 128],
                                xbfs[t][:, dc * 128 : (dc + 1) * 128],
                                ident[:],
                            )
                        nc.scalar.copy(xT[:, d0:d1, t, :], tp[:])

            # w_v wave (needed by the v phase, ~18us in)
            for kc in range(DC):
                wstv = wstage.tile([128, D], F32, tag="wstv", name="wstv")
                nc.sync.dma_start(
                    wstv[:], wqkv_d.ap()[kc * 128 : (kc + 1) * 128, 2 * D :]
                )
                nc.scalar.copy(wqk[:, kc, 2 * D :], wstv[:])

            with tc.tile_pool(name="ps_qk", bufs=2, space="PSUM") as ps_qk:

                def emit_qkT_pair(hp, nhs=(0, 1)):
                    for ct in (hp, CT // 2 + hp):
                        for nh in nhs:
                            s_qk = ps_qk.tile([128, 512], F32, tag="s",
                                              name="s_qk")
                            for kc in range(DC):
                                nc.tensor.matmul(
                                    s_qk[:],
                                    wqk[:, kc, ct * 128 : (ct + 1) * 128],
                                    xT[:, kc, nh * 4 : (nh + 1) * 4, :],
                                    start=(kc == 0),
                                    stop=(kc == DC - 1),
                                )
                            dst = qkT[:, ct, nh * 512 : (nh + 1) * 512]
                            if ct < CT // 2:  # q: (psum + b) * 1/sqrt(hd)
                                nc.vector.tensor_scalar(
                                    dst, s_qk[:], bqk_sb[:, ct : ct + 1],
                                    SCALE, op0=ADD, op1=MULT,
                                )
                            else:  # k: psum + b
                                nc.vector.tensor_scalar(
                                    dst, s_qk[:], bqk_sb[:, ct : ct + 1],
                                    None, op0=ADD,
                                )

                with tc.tile_pool(name="ps_tp", bufs=2,
                                  space="PSUM") as ps_tp:
                    emit_transposes(ps_tp, range(NT))
                emit_qkT_pair(0, nhs=(0,))
                emit_qkT_pair(0, nhs=(1,))

                def emit_bv_fold():
                    # Fold the attention v-bias into the proj bias:
                    # out = (aT + 1 (x) b_v).T @ wp + b_proj
                    #     = aT.T @ wp + (b_v @ wp + b_proj),
                    # so the per-head +b_v pass on aT disappears. b_v @ wp
                    # is a [1,768] row computed on the PE (two ps_qk slots),
                    # then bproj_bc is rebuilt via a DRAM broadcast.
                    bvbf = consts.tile([128, DC], BF16, name="bvbf")
                    nc.vector.tensor_copy(bvbf[:], bv_sb[:])
                    psA = ps_qk.tile([128, 512], F32, tag="s", name="psbvA")
                    psB = ps_qk.tile([128, 512], F32, tag="s", name="psbvB")
                    for kc in range(DC):
                        for ps, j0, j1 in ((psA, 0, 512), (psB, 512, D)):
                            nc.tensor.matmul(
                                ps[0:1, : j1 - j0],
                                bvbf[:, kc : kc + 1],
                                wp_sb[:, kc, j0:j1],
                                start=(kc == 0),
                                stop=(kc == DC - 1),
                            )
                    bp2row = pstage.tile([128, D], F32, tag="xst",
                                         name="bp2row")
                    nc.vector.tensor_tensor(
                        bp2row[0:1, 0:512], psA[0:1, :], bproj_bc[0:1, 0:512],
                        ADD,
                    )
                    nc.vector.tensor_tensor(
                        bp2row[0:1, 512:D], psB[0:1, 0:256],
                        bproj_bc[0:1, 512:D], ADD,
                    )
                    bp2dram = drp.tile([1, D], F32, tag="bp2", name="bp2")
                    nc.sync.dma_start(bp2dram[:], bp2row[0:1, :])
                    bp2_ap = bp2dram[:]
                    nc.sync.dma_start(
                        bproj_bc[:],
                        bass.AP(tensor=bp2_ap.tensor, offset=bp2_ap.offset,
                                ap=[[0, 128]] + bp2_ap.ap[1:]),
                    )

                # remaining qk columns + w_proj land during the head loop
                for kc in range(DC):
                    for c0, c1, p0 in W2:
                        wst2 = wstage.tile([128, 512], F32, tag="wst2",
                                           name="wst2")
                        nc.sync.dma_start(
                            wst2[:],
                            wqkv_d.ap()[kc * 128 : (kc + 1) * 128, c0:c1],
                        )
                        nc.scalar.copy(wqk[:, kc, c0:c1], wst2[:])
                for kc in range(DC):
                    wpst = wstage.tile([128, D], F32, tag="wstv", name="wpst")
                    nc.sync.dma_start(
                        wpst[:], wp_d.ap()[kc * 128 : (kc + 1) * 128, :]
                    )
                    nc.gpsimd.tensor_copy(wp_sb[:, kc, :], wpst[:])

                with tc.tile_pool(name="ps_sc", bufs=1, space="PSUM") as ps_sc:

                    def emit_scores(hp, jobs=None, mid=None):
                        # Both heads of the pair as adjacent 64-row-mode
                        # matmuls: par0 streams through PE tile T0 (SBUF
                        # rows 0:64), par1 through T8 (rows 64:128) into
                        # different psum banks -> they execute concurrently.
                        # The previous pair's attnv chain-groups (jobs) are
                        # interleaved between kc steps so the exp stream
                        # never waits on a monolithic attnv block.
                        expTs = {}
                        for par in range(2):
                            expTs[par] = pexp.tile(
                                [128, KC, N], BF16, tag="expT", name="expT"
                            )
                        kt = CT // 2 + hp
                        if jobs:
                            jobs[0]()
                        for kc in range(KC):
                            sc0 = ps_sc.tile([128, N], F32, tag="sc0",
                                             name="sc0")
                            sc1 = ps_sc.tile([128, N], F32, tag="sc1",
                                             name="sc1")
                            k0 = kc * 128
                            nc.tensor.matmul(
                                sc0[:, 0:512],
                                qkT[0:64, kt, k0 : k0 + 128],
                                qkT[0:64, hp, 0:512],
                                start=True, stop=True,
                            )
                            nc.tensor.matmul(
                                sc1[:, 0:512],
                                qkT[64:128, kt, k0 : k0 + 128],
                                qkT[64:128, hp, 0:512],
                                start=True, stop=True,
                            )
                            nc.tensor.matmul(
                                sc1[:, 512:1024],
                                qkT[64:128, kt, k0 : k0 + 128],
                                qkT[64:128, hp, 512:1024],
                                start=True, stop=True,
                            )
                            nc.tensor.matmul(
                                sc0[:, 512:1024],
                                qkT[0:64, kt, k0 : k0 + 128],
                                qkT[0:64, hp, 512:1024],
                                start=True, stop=True,
                            )
                            nc.scalar.activation(
                                expTs[0][:, kc, :], sc0[:], Exp
                            )
                            nc.scalar.activation(
                                expTs[1][:, kc, :], sc1[:], Exp
                            )
                            if jobs and kc < len(jobs) - 1:
                                jobs[kc + 1]()
                            if mid and kc == 4:
                                mid()
                        return expTs

                    expTs_cur = emit_scores(0)

                    # v = x @ w_v, fills the PE while pair 0's exps drain
                    with tc.tile_pool(name="ps_v", bufs=1, space="PSUM") as ps_v:
                        for t in range(NT):
                            ps = ps_v.tile([128, D], F32, tag="v")
                            for kc in range(DC):
                                for j0, j1 in ((0, 512), (512, D)):
                                    nc.tensor.matmul(
                                        ps[:, j0:j1],
                                        xT[:, kc, t, :],
                                        wqk[:, kc, 2 * D + j0 : 2 * D + j1],
                                        start=(kc == 0),
                                        stop=(kc == DC - 1),
                                    )
                            psv = ps.rearrange(
                                "p (hp two c) -> p hp two c", two=2, c=64
                            )
                            nc.vector.tensor_copy(
                                vpack[:, t, :, 0, 0:64], psv[:, :, 0, :]
                            )
                            nc.vector.tensor_copy(
                                vpack[:, t, :, 1, 64:128], psv[:, :, 1, :]
                            )

                    with tc.tile_pool(name="ps_av", bufs=2, space="PSUM") as ps_av:

                        def make_attnv_jobs(hp, expTs, pend):
                            # Four chain-groups (par x nh). Row-split
                            # accumulation: key 64-chunks alternate SBUF
                            # partition halves -> T0/T8 chains run
                            # concurrently into separate psum banks, then
                            # lo+hi combined on DVE (psum + sbuf). After a
                            # par's second group, the reciprocal broadcast
                            # (DRAM roundtrip, ~6us latency) is kicked off;
                            # the final normalize multiply is emitted later
                            # so the DVE queue never blocks on it.
                            state = {}

                            def group(par, nh):
                                def run():
                                    off = 64 * par
                                    expT = expTs[par]
                                    if nh == 0:
                                        state[par] = ppa.tile(
                                            [128, N], BF16, tag="pa_sb",
                                            name="pa_sb",
                                        )
                                    pa_sb = state[par]
                                    lo = ps_av.tile([128, 512], F32, tag="pa")
                                    hi = ps_av.tile([128, 512], F32, tag="pa")
                                    for c in range(2 * KC):
                                        half = c & 1
                                        kcc = c >> 1
                                        dst = lo if half == 0 else hi
                                        h0 = half * 64
                                        nc.tensor.matmul(
                                            dst[:],
                                            vpack[h0 : h0 + 64, kcc, hp, par, :],
                                            expT[h0 : h0 + 64, kcc,
                                                 nh * 512 : (nh + 1) * 512],
                                            start=(c < 2),
                                            stop=(c >= 2 * KC - 2),
                                        )
                                    lo_sb = plo.tile([128, 512], F32,
                                                     tag="lo_sb", name="lo_sb")
                                    nc.vector.tensor_copy(lo_sb[:], lo[:])
                                    with nc.allow_low_precision(
                                        reason="attn weights tolerate bf16"
                                    ):
                                        nc.vector.tensor_tensor(
                                            pa_sb[:, nh * 512 : (nh + 1) * 512],
                                            lo_sb[:], hi[:], ADD,
                                        )
                                    if nh != 1:
                                        return
                                    pend.append(
                                        {"hp": hp, "par": par,
                                         "pa_sb": pa_sb,
                                         "sumrow": 127 if par == 0 else 0}
                                    )
                                return run

                            return [group(0, 0), group(0, 1),
                                    group(1, 0), group(1, 1)]

                        # The normalize roundtrip is pipelined across
                        # iterations in four emission stages so that every
                        # instruction's dependency is already satisfied when
                        # its in-order engine queue reaches it:
                        #   rd (sums row -> DRAM)       next iter start
                        #   rp (DRAM -> [128,8])        next iter middle
                        #   recip + rd2                 next iter end
                        #   bc broadcast + multiply     following iter start
                        def stage_rd(pend):
                            for e in pend:
                                rdram = drp.tile([1, N], BF16, tag="rd",
                                                 name="rd")
                                nc.sync.dma_start(
                                    rdram[:],
                                    e["pa_sb"][e["sumrow"] : e["sumrow"] + 1, :],
                                )
                                e["rdram"] = rdram

                        def stage_rp(pend):
                            for e in pend:
                                rp = prp.tile([128, N // 128], BF16,
                                              tag="rp", name="rp")
                                # partition p <- contiguous q block: the DMA
                                # moves 128 contiguous chunks, not a 2-byte
                                # element scatter (which costs ~10us)
                                nc.sync.dma_start(
                                    rp[:],
                                    e["rdram"][0].rearrange("(i o) -> i o",
                                                            o=N // 128),
                                )
                                e["rp"] = rp

                        def stage_recip_rd2(pend):
                            for e in pend:
                                rp2 = prp.tile([128, N // 128], BF16,
                                               tag="rp2", name="rp2")
                                with nc.allow_low_precision(
                                    reason="softmax denom tolerates bf16"
                                ):
                                    nc.vector.reciprocal(rp2[:], e["rp"][:])
                                rdram2 = drp.tile([1, N], BF16, tag="rd2",
                                                  name="rd2")
                                nc.sync.dma_start(
                                    rdram2[0].rearrange("(i o) -> i o",
                                                        o=N // 128),
                                    rp2[:],
                                )
                                e["rdram2"] = rdram2

                        def stage_bc_mult(pend):
                            for e in pend:
                                hp, par = e["hp"], e["par"]
                                off = 64 * par
                                bc = pbc.tile([128, N], BF16, tag="bc",
                                              name="bc")
                                rd_ap = e["rdram2"][:]
                                rec_bcast = bass.AP(
                                    tensor=rd_ap.tensor,
                                    offset=rd_ap.offset,
                                    ap=[[0, 64]] + rd_ap.ap[1:],
                                )
                                nc.sync.dma_start(bc[off : off + 64, :],
                                                  rec_bcast)
                                dst = aT[off : off + 64, hp, :]
                                with nc.allow_low_precision(
                                    reason="attn weights tolerate bf16"
                                ):
                                    nc.gpsimd.tensor_tensor(
                                        dst, e["pa_sb"][off : off + 64, :],
                                        bc[off : off + 64, :], MULT,
                                    )
                            del pend[:]

                        pend_q = []
                        for hp in range(H // 2):
                            if len(pend_q) >= 2:
                                stage_bc_mult(pend_q.pop(0))
                            prev = pend_q[-1] if pend_q else None
                            if prev:
                                stage_rd(prev)
                            if hp + 1 < H // 2:
                                emit_qkT_pair(hp + 1)
                            if hp == 2:
                                emit_bv_fold()
                            cur = []
                            jobs = make_attnv_jobs(hp, expTs_cur, cur)
                            mid = (lambda p=prev: stage_rp(p)) if prev else None
                            if hp + 1 < H // 2:
                                expTs_cur = emit_scores(hp + 1, jobs=jobs,
                                                        mid=mid)
                            else:
                                # last pair: interleave both trailing pairs'
                                # normalize stages between the chain groups
                                # so aT(4) and aT(5) land with minimal tail
                                # latency (proj's kc4/kc5 are ordered last)
                                jobs[0]()
                                jobs[1]()          # par0 sums complete
                                if mid:
                                    mid()          # rp(p4)
                                e0 = cur[0:1]
                                stage_rd(e0)
                                jobs[2]()
                                stage_recip_rd2(prev)   # p4
                                stage_rp(e0)
                                jobs[3]()          # par1 sums complete
                                e1 = cur[1:2]
                                stage_rd(e1)
                                stage_bc_mult(prev)     # aT(4) done
                                stage_recip_rd2(e0)
                                stage_rp(e1)
                                stage_bc_mult(e0)
                                stage_recip_rd2(e1)
                                stage_bc_mult(e1)
                            if prev and hp + 1 < H // 2:
                                stage_recip_rd2(prev)
                            pend_q.append(cur)

            # ---------------- proj ----------------
            with tc.tile_pool(name="ps_p", bufs=4, space="PSUM") as ps_p:
                for qt in range(NT):
                    pp = ps_p.tile([128, D], F32, tag="pp")
                    for kc in range(DC):
                        for j0, j1 in ((0, 512), (512, D)):
                            nc.tensor.matmul(
                                pp[:, j0:j1],
                                aT[:, kc, qt * 128 : (qt + 1) * 128],
                                wp_sb[:, kc, j0:j1],
                                start=(kc == 0),
                                stop=(kc == DC - 1),
                            )
                    ob = pstage.tile([128, D], F32, tag="xst", name="ob")
                    nc.vector.tensor_tensor(ob[:], pp[:], bproj_bc[:], ADD)
                    oeng = nc.sync if qt % 2 == 0 else nc.scalar
                    oeng.dma_start(
                        out_d.ap()[qt * 128 : (qt + 1) * 128, :], ob[:]
                    )

    _dedupe_ldweights(nc)
    if legalize:
        _legalize_waits(nc)
    return nc


_NC_CACHE = {}
LAST_RESULT = None


def kernel(x, w_qkv, b_qkv, w_proj, b_proj):
    global LAST_RESULT
    x = np.ascontiguousarray(np.asarray(x, dtype=np.float32))
    w_qkv = np.ascontiguousarray(np.asarray(w_qkv, dtype=np.float32))
    b_qkv = np.ascontiguousarray(np.asarray(b_qkv, dtype=np.float32))
    w_proj = np.ascontiguousarray(np.asarray(w_proj, dtype=np.float32))
    b_proj = np.ascontiguousarray(np.asarray(b_proj, dtype=np.float32))
    B = x.shape[0]
    assert x.shape == (B, N, D) and B == 8

    if "nc" not in _NC_CACHE:
        _NC_CACHE["nc"] = build()
    nc = _NC_CACHE["nc"]

    in_maps = [
        {"x": x[i], "w_qkv": w_qkv, "b_qkv": b_qkv,
         "w_proj": w_proj, "b_proj": b_proj}
        for i in range(B)
    ]
    trace = bool(int(os.environ.get("KERNEL_TRACE", "0")))
    res = run_bass_kernel_spmd(
        nc, in_maps, core_ids=list(range(8)), trace=trace
    )
    LAST_RESULT = res
    return np.stack([res.results[i]["out"] for i in range(B)], axis=0)


# revision 70
# speedup vs baseline: 1.0006x; 1.0006x over previous
"""Multi-head attention (B=8, N=1024, D=768, H=12) on 8 TRN2 NeuronCores.

Data-parallel: one batch element per core, no collectives. Per-core kernel:
  xT = x.T                          (PE transpose, bf16)
  qkT[c, t] = w_qkv[:, c].T @ xT    (c in [0, 1536): q rows then k rows,
                                     bias fused, q pre-scaled by 1/8)
  v[t, c]   = xT.T @ w_qkv[:, 1536:]  (bias folded later: softmax rows
                                       sum to 1 so attn @ (v+b) = attn@v + b)
  per head pair: scoresT[k, q] = kT.T @ qT for both heads as ADJACENT
            64-row-mode matmuls (PE tiles T0/T8 run concurrently, ~2x);
            exp over [128,1024] psum in one wide ACTIVATE per (kc, par);
            aT+sums = vpack.T @ expT row-split into T0/T8 accumulation
            chains (key 64-chunks alternate SBUF partition halves), then
            lo+hi combined on DVE; ones column in vpack emits softmax
            denominators; aT = (lo+hi) * (1/sums) + b_v.
  out = aT.T @ w_proj + b_proj

All matmul operands bf16 (PSUM fp32, softmax fp32). Engine balance:
exp stream ~97us on scalar, PE ~110us (scores/attnv at 2x via array
tiling), DVE ~100us, staging casts on GPSIMD.
"""

import os

import numpy as np

import bass_rust
from bass_rust import ScopedClock

import concourse.bass as bass
import concourse.tile as tile
from concourse import mybir
from concourse.bass_utils import run_bass_kernel_spmd
from concourse.masks import make_identity

# ---------------------------------------------------------------------------
# Workarounds: this container's walrus allows only ONE sync wait per
# instruction ("Too many sync wait commands"). Split extras onto same-engine
# NoOps (engine sequencers execute in program order).
# ---------------------------------------------------------------------------
_MAX_WAITS = 1


def _patched_drain_and_barrier(self, tick_clock, wait_clock):
    nc = self.nc
    drain_inst = nc.sync.drain()
    wait_clock.add_sem_waits(
        drain_inst.ins, ScopedClock({None: tick_clock.global_clock})
    )
    waits = list(drain_inst.ins.sync_info.on_wait)
    if len(waits) > _MAX_WAITS:
        drain_inst.ins.sync_info = bass_rust.SyncInfo(on_wait=[], on_update=[])
        by_num = {h.num: h for h in self.sems.allocated().values()}
        for w in waits:
            h = by_num.get(w.id)
            if h is None:
                h = bass_rust.SemaphoreHandle(name=w.ant_name, num=w.id)
            nc.sync.wait_ge(h, w.wait_value)

    nc.all_engine_barrier()
    assert self.sems is not None
    popped = nc._tile_sem_poison_stack.pop()
    assert popped is self._sem_poison
    nc.clear_and_free_semaphores(list(self.sems.allocated().values()))
    nc.all_engine_barrier()


tile.TileContext._drain_and_barrier = _patched_drain_and_barrier


def _legalize_waits(nc):
    n_split = 0
    for fn in nc.m.functions:
        for bb in fn.blocks:
            insts = bb.instructions
            if not any(
                i.sync_info is not None and len(i.sync_info.on_wait) > _MAX_WAITS
                for i in insts
            ):
                continue
            new = []
            for inst in insts:
                si = inst.sync_info
                if si is not None and len(si.on_wait) > _MAX_WAITS:
                    waits = list(si.on_wait)
                    keep, extra = waits[:_MAX_WAITS], waits[_MAX_WAITS:]
                    for j, w in enumerate(extra):
                        nop = mybir.InstNoOp(
                            name=f"{inst.name}-ws{j}", ins=[], outs=[],
                            engine=inst.engine,
                        )
                        nop.sync_info = bass_rust.SyncInfo(on_wait=[w], on_update=[])
                        new.append(nop)
                        n_split += 1
                    inst.sync_info = bass_rust.SyncInfo(
                        on_wait=keep, on_update=list(si.on_update)
                    )
                new.append(inst)
            bb.instructions = new
    return n_split


def _dedupe_ldweights(nc):
    """Drop an InstLdweights when the PE already holds identical weights
    (same physical AP + mode). bass emits one LDW per matmul; back-to-back
    matmuls sharing a stationary operand reload it pointlessly."""
    n = 0
    for fn in nc.m.functions:
        for bb in fn.blocks:
            insts = bb.instructions
            last_key = None
            new = []
            changed = False
            for inst in insts:
                tn = type(inst).__name__
                if tn == "InstLdweights":
                    key = (
                        repr(inst.ins[0]),
                        getattr(inst, "perf_mode", None),
                        getattr(inst, "is_transpose", None),
                        getattr(inst, "tile_position", None),
                    )
                    if key == last_key:
                        si = inst.sync_info
                        if si is not None and (si.on_wait or si.on_update):
                            nop = mybir.InstNoOp(
                                name=inst.name + "-dw", ins=[], outs=[],
                                engine=inst.engine,
                            )
                            nop.sync_info = si
                            new.append(nop)
                        n += 1
                        changed = True
                        continue
                    last_key = key
                new.append(inst)
            if changed:
                bb.instructions = new
    return n


# ---------------------------------------------------------------------------
# Kernel builder (per-core shapes hardcoded: x [1024, 768])
# ---------------------------------------------------------------------------
N, D, H, HD = 1024, 768, 12, 64
NT = N // 128       # 8 token chunks
DC = D // 128       # 6 d chunks
CT = (2 * D) // 128  # 12 qk col tiles
KC = N // 128       # 8 key chunks
SCALE = HD ** -0.5

F32 = mybir.dt.float32
BF16 = mybir.dt.bfloat16
Exp = mybir.ActivationFunctionType.Exp
ADD = mybir.AluOpType.add
MULT = mybir.AluOpType.mult


def build(legalize=True):
    nc = bass.Bass()
    x_d = nc.declare_dram_parameter("x", [N, D], F32, isOutput=False)
    wqkv_d = nc.declare_dram_parameter("w_qkv", [D, 3 * D], F32, isOutput=False)
    bqkv_d = nc.declare_dram_parameter("b_qkv", [3 * D], F32, isOutput=False)
    wp_d = nc.declare_dram_parameter("w_proj", [D, D], F32, isOutput=False)
    bp_d = nc.declare_dram_parameter("b_proj", [D], F32, isOutput=False)
    out_d = nc.declare_dram_parameter("out", [N, D], F32, isOutput=True)

    with tile.TileContext(nc) as tc:
        with (
            tc.tile_pool(name="persist", bufs=1) as persist,
            tc.tile_pool(name="consts", bufs=1) as consts,
            tc.tile_pool(name="pstage", bufs=3) as pstage,
            tc.tile_pool(name="wstage", bufs=2) as wstage,
            tc.tile_pool(name="ppa", bufs=4) as ppa,
            tc.tile_pool(name="prp", bufs=4) as prp,
            tc.tile_pool(name="pbc", bufs=4) as pbc,
            tc.tile_pool(name="plo", bufs=1) as plo,
            tc.tile_pool(name="pexp", bufs=3) as pexp,
            tc.tile_pool(name="drp", bufs=2, space="DRAM") as drp,
        ):
            qkT = persist.tile([128, CT, N], BF16)         # [qk col, tok]
            vpack = persist.tile([128, KC, 6, 2, 128], BF16)
            aT = persist.tile([128, DC, N], BF16)          # [d, tok]
            wp_sb = persist.tile([128, DC, D], BF16)
            xT = persist.tile([128, DC, NT, 128], BF16)    # [d, tok]
            wqk = persist.tile([128, DC, 3 * D], BF16)
            bqk_sb = consts.tile([128, CT], F32)
            bv_sb = consts.tile([128, DC], F32)
            bproj_bc = consts.tile([128, D], F32)
            ident = consts.tile([128, 128], BF16)

            make_identity(nc, ident[:])

            # ---- load phase: x half 0 + pair-0 qk weight cols first ----
            # (critical path to the first exp), then x half 1, w_v, the
            # remaining qk columns and w_proj (all land under compute).
            WQK0 = ((0, 256, 0), (768, 1024, 256))         # pair-0 q,k cols
            W2 = ((256, 768, 0), (1024, 1536, 512))        # pairs 1-5 cols

            xbfs = []

            def load_x_chunk(i):
                xst = pstage.tile([128, D], F32, tag="xst")
                nc.sync.dma_start(
                    xst[:], x_d.ap()[i * 128 : (i + 1) * 128, :]
                )
                xbf = pstage.tile([128, D], BF16, tag="xbf")
                nc.vector.tensor_copy(xbf[:], xst[:])
                xbfs.append(xbf)

            for i in range(4):
                load_x_chunk(i)
            wq_ap = wqkv_d.ap()
            for kc in range(DC):
                wst0 = wstage.tile([128, 512], F32, tag="wst0", name="wst0")
                # q cols 0:256 + k cols 768:1024 in one 2-block strided DMA
                nc.sync.dma_start(
                    wst0[:],
                    bass.AP(tensor=wq_ap.tensor,
                            offset=wq_ap.offset + kc * 128 * 3 * D,
                            ap=[[3 * D, 128], [768, 2], [1, 256]]),
                )
                for c0, c1, p0 in WQK0:
                    nc.vector.tensor_copy(
                        wqk[:, kc, c0:c1], wst0[:, p0 : p0 + (c1 - c0)]
                    )
            for i in range(4, NT):
                load_x_chunk(i)

            nc.sync.dma_start(
                bqk_sb[:], bqkv_d.ap()[: 2 * D].rearrange("(o i) -> i o", i=128)
            )
            nc.sync.dma_start(
                bv_sb[:], bqkv_d.ap()[2 * D :].rearrange("(o i) -> i o", i=128)
            )
            bp_ap = bp_d.ap()
            nc.sync.dma_start(
                bproj_bc[:],
                bass.AP(tensor=bp_ap.tensor, offset=bp_ap.offset,
                        ap=[[0, 128]] + bp_ap.ap),
            )
            # ones columns of vpack (even head: col 127; odd head: col 0);
            # the v-value columns are fully overwritten by the v copies and
            # the remaining columns feed only unread psum partitions.
            nc.vector.memset(vpack[:, :, :, 0, 127:128], 1.0)
            nc.vector.memset(vpack[:, :, :, 1, 0:1], 1.0)

            # x transposes (PE warms up here); batched psum->sbuf copies on
            # the scalar engine, which is idle until the first exp.
            def emit_transposes(ps_tp, ts):
                for t in ts:
                    for d0, d1 in ((0, 4), (4, 6)):
                        tp = ps_tp.tile([128, (d1 - d0) * 128], BF16, tag="tp")
                        for j, dc in enumerate(range(d0, d1)):
                            nc.tensor.transpose(
                                tp[:, j * 128 : (j + 1) * 128],
                                xbfs[t][:, dc * 128 : (dc + 1) * 128],
                                ident[:],
                            )
                        nc.scalar.copy(xT[:, d0:d1, t, :], tp[:])

            # w_v wave (needed by the v phase, ~18us in)
            for kc in range(DC):
                wstv = wstage.tile([128, D], F32, tag="wstv", name="wstv")
                nc.sync.dma_start(
                    wstv[:], wqkv_d.ap()[kc * 128 : (kc + 1) * 128, 2 * D :]
                )
                nc.scalar.copy(wqk[:, kc, 2 * D :], wstv[:])

            with tc.tile_pool(name="ps_qk", bufs=2, space="PSUM") as ps_qk:

                def emit_qkT_pair(hp, nhs=(0, 1)):
                    for ct in (hp, CT // 2 + hp):
                        for nh in nhs:
                            s_qk = ps_qk.tile([128, 512], F32, tag="s",
                                              name="s_qk")
                            for kc in range(DC):
                                nc.tensor.matmul(
                                    s_qk[:],
                                    wqk[:, kc, ct * 128 : (ct + 1) * 128],
                                    xT[:, kc, nh * 4 : (nh + 1) * 4, :],
                                    start=(kc == 0),
                                    stop=(kc == DC - 1),
                                )
                            dst = qkT[:, ct, nh * 512 : (nh + 1) * 512]
                            if ct < CT // 2:  # q: (psum + b) * 1/sqrt(hd)
                                nc.vector.tensor_scalar(
                                    dst, s_qk[:], bqk_sb[:, ct : ct + 1],
                                    SCALE, op0=ADD, op1=MULT,
                                )
                            else:  # k: psum + b
                                nc.vector.tensor_scalar(
                                    dst, s_qk[:], bqk_sb[:, ct : ct + 1],
                                    None, op0=ADD,
                                )

                with tc.tile_pool(name="ps_tp", bufs=2,
                                  space="PSUM") as ps_tp:
                    emit_transposes(ps_tp, range(NT))
                emit_qkT_pair(0, nhs=(0,))
                emit_qkT_pair(0, nhs=(1,))

                def emit_bv_fold():
                    # Fold the attention v-bias into the proj bias:
                    # out = (aT + 1 (x) b_v).T @ wp + b_proj
                    #     = aT.T @ wp + (b_v @ wp + b_proj),
                    # so the per-head +b_v pass on aT disappears. b_v @ wp
                    # is a [1,768] row computed on the PE (two ps_qk slots),
                    # then bproj_bc is rebuilt via a DRAM broadcast.
                    bvbf = consts.tile([128, DC], BF16, name="bvbf")
                    nc.vector.tensor_copy(bvbf[:], bv_sb[:])
                    psA = ps_qk.tile([128, 512], F32, tag="s", name="psbvA")
                    psB = ps_qk.tile([128, 512], F32, tag="s", name="psbvB")
                    for kc in range(DC):
                        for ps, j0, j1 in ((psA, 0, 512), (psB, 512, D)):
                            nc.tensor.matmul(
                                ps[0:1, : j1 - j0],
                                bvbf[:, kc : kc + 1],
                                wp_sb[:, kc, j0:j1],
                                start=(kc == 0),
                                stop=(kc == DC - 1),
                            )
                    bp2row = pstage.tile([128, D], F32, tag="xst",
                                         name="bp2row")
                    nc.vector.tensor_tensor(
                        bp2row[0:1, 0:512], psA[0:1, :], bproj_bc[0:1, 0:512],
                        ADD,
                    )
                    nc.vector.tensor_tensor(
                        bp2row[0:1, 512:D], psB[0:1, 0:256],
                        bproj_bc[0:1, 512:D], ADD,
                    )
                    bp2dram = drp.tile([1, D], F32, tag="bp2", name="bp2")
                    nc.sync.dma_start(bp2dram[:], bp2row[0:1, :])
                    bp2_ap = bp2dram[:]
                    nc.sync.dma_start(
                        bproj_bc[:],
                        bass.AP(tensor=bp2_ap.tensor, offset=bp2_ap.offset,
                                ap=[[0, 128]] + bp2_ap.ap[1:]),
                    )

                # remaining qk columns + w_proj land during the head loop
                for kc in range(DC):
                    for c0, c1, p0 in W2:
                        wst2 = wstage.tile([128, 512], F32, tag="wst2",
                                           name="wst2")
                        nc.sync.dma_start(
                            wst2[:],
                            wqkv_d.ap()[kc * 128 : (kc + 1) * 128, c0:c1],
                        )
                        nc.vector.tensor_copy(wqk[:, kc, c0:c1], wst2[:])
                for kc in range(DC):
                    wpst = wstage.tile([128, D], F32, tag="wstv", name="wpst")
                    nc.sync.dma_start(
                        wpst[:], wp_d.ap()[kc * 128 : (kc + 1) * 128, :]
                    )
                    nc.gpsimd.tensor_copy(wp_sb[:, kc, :], wpst[:])

                with tc.tile_pool(name="ps_sc", bufs=1, space="PSUM") as ps_sc:

                    def emit_scores(hp, jobs=None, mid=None):
                        # Both heads of the pair as adjacent 64-row-mode
                        # matmuls: par0 streams through PE tile T0 (SBUF
                        # rows 0:64), par1 through T8 (rows 64:128) into
                        # different psum banks -> they execute concurrently.
                        # The previous pair's attnv chain-groups (jobs) are
                        # interleaved between kc steps so the exp stream
                        # never waits on a monolithic attnv block.
                        expTs = {}
                        for par in range(2):
                            expTs[par] = pexp.tile(
                                [128, KC, N], BF16, tag="expT", name="expT"
                            )
                        kt = CT // 2 + hp
                        if jobs:
                            jobs[0]()
                        for kc in range(KC):
                            sc0 = ps_sc.tile([128, N], F32, tag="sc0",
                                             name="sc0")
                            sc1 = ps_sc.tile([128, N], F32, tag="sc1",
                                             name="sc1")
                            k0 = kc * 128
                            nc.tensor.matmul(
                                sc0[:, 0:512],
                                qkT[0:64, kt, k0 : k0 + 128],
                                qkT[0:64, hp, 0:512],
                                start=True, stop=True,
                            )
                            nc.tensor.matmul(
                                sc1[:, 0:512],
                                qkT[64:128, kt, k0 : k0 + 128],
                                qkT[64:128, hp, 0:512],
                                start=True, stop=True,
                            )
                            nc.tensor.matmul(
                                sc1[:, 512:1024],
                                qkT[64:128, kt, k0 : k0 + 128],
                                qkT[64:128, hp, 512:1024],
                                start=True, stop=True,
                            )
                            nc.tensor.matmul(
                                sc0[:, 512:1024],
                                qkT[0:64, kt, k0 : k0 + 128],
                                qkT[0:64, hp, 512:1024],
                                start=True, stop=True,
                            )
                            nc.scalar.activation(
                                expTs[0][:, kc, :], sc0[:], Exp
                            )
                            nc.scalar.activation(
                                expTs[1][:, kc, :], sc1[:], Exp
                            )
                            if jobs and kc < len(jobs) - 1:
                                jobs[kc + 1]()
                            if mid and kc == 4:
                                mid()
                        return expTs

                    expTs_cur = emit_scores(0)

                    # v = x @ w_v, fills the PE while pair 0's exps drain
                    with tc.tile_pool(name="ps_v", bufs=1, space="PSUM") as ps_v:
                        for t in range(NT):
                            ps = ps_v.tile([128, D], F32, tag="v")
                            for kc in range(DC):
                                for j0, j1 in ((0, 512), (512, D)):
                                    nc.tensor.matmul(
                                        ps[:, j0:j1],
                                        xT[:, kc, t, :],
                                        wqk[:, kc, 2 * D + j0 : 2 * D + j1],
                                        start=(kc == 0),
                                        stop=(kc == DC - 1),
                                    )
                            psv = ps.rearrange(
                                "p (hp two c) -> p hp two c", two=2, c=64
                            )
                            nc.vector.tensor_copy(
                                vpack[:, t, :, 0, 0:64], psv[:, :, 0, :]
                            )
                            nc.vector.tensor_copy(
                                vpack[:, t, :, 1, 64:128], psv[:, :, 1, :]
                            )

                    with tc.tile_pool(name="ps_av", bufs=2, space="PSUM") as ps_av:

                        def make_attnv_jobs(hp, expTs, pend):
                            # Four chain-groups (par x nh). Row-split
                            # accumulation: key 64-chunks alternate SBUF
                            # partition halves -> T0/T8 chains run
                            # concurrently into separate psum banks, then
                            # lo+hi combined on DVE (psum + sbuf). After a
                            # par's second group, the reciprocal broadcast
                            # (DRAM roundtrip, ~6us latency) is kicked off;
                            # the final normalize multiply is emitted later
                            # so the DVE queue never blocks on it.
                            state = {}

                            def group(par, nh):
                                def run():
                                    off = 64 * par
                                    expT = expTs[par]
                                    if nh == 0:
                                        state[par] = ppa.tile(
                                            [128, N], BF16, tag="pa_sb",
                                            name="pa_sb",
                                        )
                                    pa_sb = state[par]
                                    lo = ps_av.tile([128, 512], F32, tag="pa")
                                    hi = ps_av.tile([128, 512], F32, tag="pa")
                                    for c in range(2 * KC):
                                        half = c & 1
                                        kcc = c >> 1
                                        dst = lo if half == 0 else hi
                                        h0 = half * 64
                                        nc.tensor.matmul(
                                            dst[:],
                                            vpack[h0 : h0 + 64, kcc, hp, par, :],
                                            expT[h0 : h0 + 64, kcc,
                                                 nh * 512 : (nh + 1) * 512],
                                            start=(c < 2),
                                            stop=(c >= 2 * KC - 2),
                                        )
                                    lo_sb = plo.tile([128, 512], F32,
                                                     tag="lo_sb", name="lo_sb")
                                    nc.vector.tensor_copy(lo_sb[:], lo[:])
                                    with nc.allow_low_precision(
                                        reason="attn weights tolerate bf16"
                                    ):
                                        nc.vector.tensor_tensor(
                                            pa_sb[:, nh * 512 : (nh + 1) * 512],
                                            lo_sb[:], hi[:], ADD,
                                        )
                                    if nh != 1:
                                        return
                                    pend.append(
                                        {"hp": hp, "par": par,
                                         "pa_sb": pa_sb,
                                         "sumrow": 127 if par == 0 else 0}
                                    )
                                return run

                            return [group(0, 0), group(0, 1),
                                    group(1, 0), group(1, 1)]

                        # The normalize roundtrip is pipelined across
                        # iterations in four emission stages so that every
                        # instruction's dependency is already satisfied when
                        # its in-order engine queue reaches it:
                        #   rd (sums row -> DRAM)       next iter start
                        #   rp (DRAM -> [128,8])        next iter middle
                        #   recip + rd2                 next iter end
                        #   bc broadcast + multiply     following iter start
                        def stage_rd(pend):
                            for e in pend:
                                rdram = drp.tile([1, N], BF16, tag="rd",
                                                 name="rd")
                                nc.sync.dma_start(
                                    rdram[:],
                                    e["pa_sb"][e["sumrow"] : e["sumrow"] + 1, :],
                                )
                                e["rdram"] = rdram

                        def stage_rp(pend):
                            for e in pend:
                                rp = prp.tile([128, N // 128], BF16,
                                              tag="rp", name="rp")
                                # partition p <- contiguous q block: the DMA
                                # moves 128 contiguous chunks, not a 2-byte
                                # element scatter (which costs ~10us)
                                nc.sync.dma_start(
                                    rp[:],
                                    e["rdram"][0].rearrange("(i o) -> i o",
                                                            o=N // 128),
                                )
                                e["rp"] = rp

                        def stage_recip_rd2(pend):
                            for e in pend:
                                rp2 = prp.tile([128, N // 128], BF16,
                                               tag="rp2", name="rp2")
                                with nc.allow_low_precision(
                                    reason="softmax denom tolerates bf16"
                                ):
                                    nc.vector.reciprocal(rp2[:], e["rp"][:])
                                rdram2 = drp.tile([1, N], BF16, tag="rd2",
                                                  name="rd2")
                                nc.sync.dma_start(
                                    rdram2[0].rearrange("(i o) -> i o",
                                                        o=N // 128),
                                    rp2[:],
                                )
                                e["rdram2"] = rdram2

                        def stage_bc_mult(pend):
                            for e in pend:
                                hp, par = e["hp"], e["par"]
                                off = 64 * par
                                bc = pbc.tile([128, N], BF16, tag="bc",
                                              name="bc")
                                rd_ap = e["rdram2"][:]
                                rec_bcast = bass.AP(
                                    tensor=rd_ap.tensor,
                                    offset=rd_ap.offset,
                                    ap=[[0, 64]] + rd_ap.ap[1:],
                                )
                                nc.sync.dma_start(bc[off : off + 64, :],
                                                  rec_bcast)
                                dst = aT[off : off + 64, hp, :]
                                with nc.allow_low_precision(
                                    reason="attn weights tolerate bf16"
                                ):
                                    nc.gpsimd.tensor_tensor(
                                        dst, e["pa_sb"][off : off + 64, :],
                                        bc[off : off + 64, :], MULT,
                                    )
                            del pend[:]

                        pend_q = []
                        for hp in range(H // 2):
                            if len(pend_q) >= 2:
                                stage_bc_mult(pend_q.pop(0))
                            prev = pend_q[-1] if pend_q else None
                            if prev:
                                stage_rd(prev)
                            if hp + 1 < H // 2:
                                emit_qkT_pair(hp + 1)
                            if hp == 2:
                                emit_bv_fold()
                            cur = []
                            jobs = make_attnv_jobs(hp, expTs_cur, cur)
                            mid = (lambda p=prev: stage_rp(p)) if prev else None
                            if hp + 1 < H // 2:
                                expTs_cur = emit_scores(hp + 1, jobs=jobs,
                                                        mid=mid)
                            else:
                                # last pair: interleave both trailing pairs'
                                # normalize stages between the chain groups
                                # so aT(4) and aT(5) land with minimal tail
                                # latency (proj's kc4/kc5 are ordered last)
                                jobs[0]()
                                jobs[1]()          # par0 sums complete
                                if mid:
                                    mid()          # rp(p4)
                                e0 = cur[0:1]
                                stage_rd(e0)
                                jobs[2]()
                                stage_recip_rd2(prev)   # p4
                                stage_rp(e0)
                                jobs[3]()          # par1 sums complete
                                e1 = cur[1:2]
                                stage_rd(e1)
                                stage_bc_mult(prev)     # aT(4) done
                                stage_recip_rd2(e0)
                                stage_rp(e1)
                                stage_bc_mult(e0)
                                stage_recip_rd2(e1)
                                stage_bc_mult(e1)
                            if prev and hp + 1 < H // 2:
                                stage_recip_rd2(prev)
                            pend_q.append(cur)

            # ---------------- proj ----------------
            with tc.tile_pool(name="ps_p", bufs=4, space="PSUM") as ps_p:
                for qt in range(NT):
                    pp = ps_p.tile([128, D], F32, tag="pp")
                    for kc in range(DC):
                        for j0, j1 in ((0, 512), (512, D)):
                            nc.tensor.matmul(
                                pp[:, j0:j1],
                                aT[:, kc, qt * 128 : (qt + 1) * 128],
                                wp_sb[:, kc, j0:j1],
                                start=(kc == 0),
                                stop=(kc == DC - 1),
                            )
                    ob = pstage.tile([128, D], F32, tag="xst", name="ob")
                    nc.vector.tensor_tensor(ob[:], pp[:], bproj_bc[:], ADD)
                    oeng = nc.sync if qt % 2 == 0 else nc.scalar
                    oeng.dma_start(
                        out_d.ap()[qt * 128 : (qt + 1) * 128, :], ob[:]
                    )

    _dedupe_ldweights(nc)
    if legalize:
        _legalize_waits(nc)
    return nc


_NC_CACHE = {}
LAST_RESULT = None


def kernel(x, w_qkv, b_qkv, w_proj, b_proj):
    global LAST_RESULT
    x = np.ascontiguousarray(np.asarray(x, dtype=np.float32))
    w_qkv = np.ascontiguousarray(np.asarray(w_qkv, dtype=np.float32))
    b_qkv = np.ascontiguousarray(np.asarray(b_qkv, dtype=np.float32))
    w_proj = np.ascontiguousarray(np.asarray(w_proj, dtype=np.float32))
    b_proj = np.ascontiguousarray(np.asarray(b_proj, dtype=np.float32))
    B = x.shape[0]
    assert x.shape == (B, N, D) and B == 8

    if "nc" not in _NC_CACHE:
        _NC_CACHE["nc"] = build()
    nc = _NC_CACHE["nc"]

    in_maps = [
        {"x": x[i], "w_qkv": w_qkv, "b_qkv": b_qkv,
         "w_proj": w_proj, "b_proj": b_proj}
        for i in range(B)
    ]
    trace = bool(int(os.environ.get("KERNEL_TRACE", "0")))
    res = run_bass_kernel_spmd(
        nc, in_maps, core_ids=list(range(8)), trace=trace
    )
    LAST_RESULT = res
    return np.stack([res.results[i]["out"] for i in range(B)], axis=0)
